# Initial kernel scaffold
#
"""Squared euclidean distance kernel for Trainium2 (8 NeuronCores, SPMD).

dist[n, m] = ||mat_1[n]||^2 + ||mat_2[m]||^2 - 2 <mat_1[n], mat_2[m]>

Strategy: data-parallel shard of mat_1 rows across 8 cores; mat_2 replicated.
The whole computation is a single TensorE matmul per output tile with an
augmented contract dimension (K = 64 + 4):

    lhsT = [mat_1^T ; sq1_hi ; sq1_lo ; 1 ; 1]          (per core, [68, 12544] bf16)
    rhs  = [-2*mat_2^T ; 1 ; 1 ; sq2_hi ; sq2_lo]       (replicated, [68, 2048] bf16)

so PSUM accumulates the final distance in f32 directly (the squared norms are
carried as bf16 hi/lo pairs, recovering ~f32 accuracy for the norm terms).
The kernel is output-DMA bound: 103 MB of f32 distances per core.
"""

import numpy as np
import ml_dtypes

import concourse.bass as bass
import concourse.mybir as mybir
from concourse.tile import TileContext
from concourse.bass_utils import run_bass_kernel_spmd

N1, D, N2 = 100000, 64, 2048
NCORES = 8
ROWS_VALID = N1 // NCORES          # 12500 rows of mat_1 per core
CHUNK = 128                        # output rows per tile (PE partition dim)
NCHUNK = (ROWS_VALID + CHUNK - 1) // CHUNK   # 98
ROWS = CHUNK * NCHUNK              # 12544 (padded)
K = D + 4                          # 68: 64 features + sq1_hi/lo + ones
BANK = 512                         # fp32 PSUM bank width (max matmul free dim)
BF16 = ml_dtypes.bfloat16

_CACHE = {}


def _split_drain_waits(nc):
    """Walrus in this toolchain only accepts one sync-wait on an InstDrain;
    Tile's kernel-tail drain can carry several (one per DMA lane). Hoist the
    extras onto dedicated NoOps right before the drain."""
    for f in nc.m.functions:
        for bb in f.blocks:
            new = []
            for inst in bb.instructions:
                si = getattr(inst, "sync_info", None)
                if (
                    si is not None
                    and si.on_wait is not None
                    and len(si.on_wait) > 1
                    and type(inst).__name__ == "InstDrain"
                ):
                    for w in si.on_wait[:-1]:
                        nop = mybir.InstNoOp(
                            name=nc.get_next_instruction_name(), ins=[], outs=[]
                        )
                        nop.engine = inst.engine
                        nop.sync_info = mybir.SyncInfo(on_wait=[w], on_update=[])
                        new.append(nop)
                    si.on_wait = [si.on_wait[-1]]
                new.append(inst)
            bb.instructions[:] = new


def build_nc(rows=ROWS, n2=N2, out_bufs=6, lhs_splits=8):
    """Build the per-core Bass program (SPMD: same program on all 8 cores)."""
    nchunk = rows // CHUNK
    nbank = n2 // BANK
    half = (nbank // 2) * BANK     # DVE copies [0:half), ACT copies [half:n2)

    nc = bass.Bass()
    lhst = nc.dram_tensor("lhst", [K, rows], mybir.dt.bfloat16, kind="ExternalInput")
    rhs = nc.dram_tensor("rhs", [K, n2], mybir.dt.bfloat16, kind="ExternalInput")
    out = nc.dram_tensor("out", [rows, n2], mybir.dt.float32, kind="ExternalOutput")

    with TileContext(nc) as tc:
        with tc.tile_pool(name="const", bufs=1) as cpool, \
             tc.tile_pool(name="outp", bufs=out_bufs) as opool, \
             tc.tile_pool(name="psum", bufs=2, space="PSUM") as ppool:
            # Replicated rhs and the full per-core lhsT live in SBUF for the
            # whole kernel. lhsT is DMA'd in column-range pieces so early
            # chunks don't wait on the full 1.7 MB transfer. SWDGE (gpsimd)
            # keeps the HWDGE ring free for the output stream.
            rhs_sb = cpool.tile([K, n2], mybir.dt.bfloat16)
            nc.gpsimd.dma_start(out=rhs_sb[:], in_=rhs[:, :])

            lhs_sb = cpool.tile([K, rows], mybir.dt.bfloat16)
            split = max(CHUNK, rows // lhs_splits // CHUNK * CHUNK)
            for s0 in range(0, rows, split):
                s1 = min(s0 + split, rows)
                nc.gpsimd.dma_start(out=lhs_sb[:, s0:s1], in_=lhst[:, s0:s1])

            for c in range(nchunk):
                ps = ppool.tile([CHUNK, n2], mybir.dt.float32)
                w = lhs_sb[:, c * CHUNK:(c + 1) * CHUNK]
                for b in range(nbank):
                    nc.tensor.matmul(
                        ps[:, b * BANK:(b + 1) * BANK],
                        w,
                        rhs_sb[:, b * BANK:(b + 1) * BANK],
                        start=True,
                        stop=True,
                    )
                ot = opool.tile([CHUNK, n2], mybir.dt.float32)
                if half > 0:
                    nc.vector.tensor_copy(out=ot[:, :half], in_=ps[:, :half])
                if half < n2:
                    nc.scalar.copy(out=ot[:, half:], in_=ps[:, half:])
                nc.sync.dma_start(out=out[c * CHUNK:(c + 1) * CHUNK, :], in_=ot[:])

    _split_drain_waits(nc)
    return nc


def _prep_inputs(mat_1, mat_2, rows=ROWS, rows_valid=ROWS_VALID, n2=N2):
    """Host-side: shard + transpose + augment, f32 -> bf16 (hi/lo for norms)."""
    mat_1 = np.ascontiguousarray(np.asarray(mat_1, dtype=np.float32))
    mat_2 = np.ascontiguousarray(np.asarray(mat_2, dtype=np.float32))

    sq1 = np.square(mat_1, dtype=np.float32).sum(axis=1, dtype=np.float32)
    sq2 = np.square(mat_2, dtype=np.float32).sum(axis=1, dtype=np.float32)

    def hi_lo(v):
        hi = v.astype(BF16)
        lo = (v - hi.astype(np.float32)).astype(BF16)
        return hi, lo

    hi1, lo1 = hi_lo(sq1)
    hi2, lo2 = hi_lo(sq2)

    rhs = np.zeros((K, n2), dtype=BF16)
    rhs[0:D] = (-2.0 * mat_2.T).astype(BF16)
    rhs[D] = 1
    rhs[D + 1] = 1
    rhs[D + 2] = hi2
    rhs[D + 3] = lo2

    in_maps = []
    for c in range(NCORES):
        sl = slice(c * rows_valid, (c + 1) * rows_valid)
        lt = np.zeros((K, rows), dtype=BF16)
        lt[0:D, :rows_valid] = mat_1[sl].T.astype(BF16)
        lt[D, :rows_valid] = hi1[sl]
        lt[D + 1, :rows_valid] = lo1[sl]
        lt[D + 2] = 1
        lt[D + 3] = 1
        in_maps.append({"lhst": lt, "rhs": rhs})
    return in_maps


def kernel(mat_1, mat_2):
    if "nc" not in _CACHE:
        _CACHE["nc"] = build_nc()
    nc = _CACHE["nc"]
    in_maps = _prep_inputs(mat_1, mat_2)
    res = run_bass_kernel_spmd(nc, in_maps, core_ids=list(range(NCORES)))
    return np.concatenate(
        [res.results[c]["out"][:ROWS_VALID] for c in range(NCORES)], axis=0
    )


# revision 5
# speedup vs baseline: 3.6044x; 3.6044x over previous
"""Squared euclidean distance kernel for Trainium2 (8 NeuronCores, SPMD).

dist[n, m] = ||mat_1[n]||^2 + ||mat_2[m]||^2 - 2 <mat_1[n], mat_2[m]>

Strategy: data-parallel shard of mat_1 rows across 8 cores; mat_2 replicated.
The whole computation is a single TensorE matmul per output tile with an
augmented contract dimension (K = 64 + 4):

    lhsT = [mat_1^T ; sq1_hi ; sq1_lo ; 1 ; 1]          (per core, [68, 12544] bf16)
    rhs  = [-2*mat_2^T ; 1 ; 1 ; sq2_hi ; sq2_lo]       (replicated, [68, 2048] bf16)

so PSUM accumulates the final distance in f32 directly (the squared norms are
carried as bf16 hi/lo pairs, recovering ~f32 accuracy for the norm terms).
The kernel is output-DMA bound: 103 MB of f32 distances per core.
"""

import numpy as np
import ml_dtypes

import concourse.bass as bass
import concourse.mybir as mybir
from concourse.tile import TileContext
from concourse.bass_utils import run_bass_kernel_spmd

N1, D, N2 = 100000, 64, 2048
NCORES = 8
ROWS_VALID = N1 // NCORES          # 12500 rows of mat_1 per core
CHUNK = 128                        # output rows per tile (PE partition dim)
NCHUNK = (ROWS_VALID + CHUNK - 1) // CHUNK   # 98
ROWS = CHUNK * NCHUNK              # 12544 (padded)
K = D + 4                          # 68: 64 features + sq1_hi/lo + ones
BANK = 512                         # fp32 PSUM bank width (max matmul free dim)
BF16 = ml_dtypes.bfloat16

_CACHE = {}


def _split_multi_waits(nc):
    """Walrus in this toolchain only accepts one sync-wait per instruction.
    Tile's add_semaphores can attach several (one per producer). Hoist all but
    one onto dedicated NoOps immediately before the instruction on the same
    engine stream — same semantics, each carrying a single wait."""
    for f in nc.m.functions:
        for bb in f.blocks:
            new = []
            for inst in bb.instructions:
                si = getattr(inst, "sync_info", None)
                if si is not None and si.on_wait is not None and len(si.on_wait) > 1:
                    for w in si.on_wait[:-1]:
                        nop = mybir.InstNoOp(
                            name=nc.get_next_instruction_name(), ins=[], outs=[]
                        )
                        nop.engine = inst.engine
                        nop.sync_info = mybir.SyncInfo(on_wait=[w], on_update=[])
                        new.append(nop)
                    si.on_wait = [si.on_wait[-1]]
                new.append(inst)
            bb.instructions[:] = new


def build_nc(rows=ROWS, n2=N2, out_bufs=6, lhs_splits=8):
    """Build the per-core Bass program (SPMD: same program on all 8 cores)."""
    nchunk = rows // CHUNK
    nbank = n2 // BANK
    half = (nbank // 2) * BANK     # DVE copies [0:half), ACT copies [half:n2)

    nc = bass.Bass()
    lhst = nc.dram_tensor("lhst", [K, rows], mybir.dt.bfloat16, kind="ExternalInput")
    rhs = nc.dram_tensor("rhs", [K, n2], mybir.dt.bfloat16, kind="ExternalInput")
    out = nc.dram_tensor("out", [rows, n2], mybir.dt.float32, kind="ExternalOutput")

    with TileContext(nc) as tc:
        with tc.tile_pool(name="const", bufs=1) as cpool, \
             tc.tile_pool(name="outp", bufs=out_bufs) as opool, \
             tc.tile_pool(name="psum", bufs=2, space="PSUM") as ppool:
            # Replicated rhs and the full per-core lhsT live in SBUF for the
            # whole kernel. lhsT is DMA'd in column-range pieces so early
            # chunks don't wait on the full 1.7 MB transfer. SWDGE (gpsimd)
            # keeps the HWDGE ring free for the output stream.
            rhs_sb = cpool.tile([K, n2], mybir.dt.bfloat16)
            nc.gpsimd.dma_start(out=rhs_sb[:], in_=rhs[:, :])

            lhs_sb = cpool.tile([K, rows], mybir.dt.bfloat16)
            split = max(CHUNK, rows // lhs_splits // CHUNK * CHUNK)
            for s0 in range(0, rows, split):
                s1 = min(s0 + split, rows)
                nc.gpsimd.dma_start(out=lhs_sb[:, s0:s1], in_=lhst[:, s0:s1])

            for c in range(nchunk):
                ps = ppool.tile([CHUNK, n2], mybir.dt.float32)
                w = lhs_sb[:, c * CHUNK:(c + 1) * CHUNK]
                for b in range(nbank):
                    nc.tensor.matmul(
                        ps[:, b * BANK:(b + 1) * BANK],
                        w,
                        rhs_sb[:, b * BANK:(b + 1) * BANK],
                        start=True,
                        stop=True,
                    )
                ot = opool.tile([CHUNK, n2], mybir.dt.float32)
                if half > 0:
                    nc.vector.tensor_copy(out=ot[:, :half], in_=ps[:, :half])
                if half < n2:
                    nc.scalar.copy(out=ot[:, half:], in_=ps[:, half:])
                nc.sync.dma_start(out=out[c * CHUNK:(c + 1) * CHUNK, :], in_=ot[:])

    _split_multi_waits(nc)
    return nc


def build_timing_nc(rows=ROWS, n2=N2, out_bufs=6, lhs_splits=8, repeats=8):
    """Same pipeline, repeated `repeats` times via a hardware For loop, with
    the big output going to internal DRAM scratch (no host transfer) and a
    tiny external output. Used only for wall-clock timing of HW exec."""
    nchunk = rows // CHUNK
    nbank = n2 // BANK
    half = (nbank // 2) * BANK

    nc = bass.Bass()
    lhst = nc.dram_tensor("lhst", [K, rows], mybir.dt.bfloat16, kind="ExternalInput")
    rhs = nc.dram_tensor("rhs", [K, n2], mybir.dt.bfloat16, kind="ExternalInput")
    out = nc.dram_tensor("scratch_out", [rows, n2], mybir.dt.float32,
                         kind="Internal")
    tout = nc.dram_tensor("tout", [1, 4], mybir.dt.float32,
                           kind="ExternalOutput")

    with TileContext(nc) as tc:
        with tc.tile_pool(name="const", bufs=1) as cpool, \
             tc.tile_pool(name="outp", bufs=out_bufs) as opool, \
             tc.tile_pool(name="psum", bufs=2, space="PSUM") as ppool:
            rhs_sb = cpool.tile([K, n2], mybir.dt.bfloat16)
            nc.gpsimd.dma_start(out=rhs_sb[:], in_=rhs[:, :])
            lhs_sb = cpool.tile([K, rows], mybir.dt.bfloat16)
            split = max(CHUNK, rows // lhs_splits // CHUNK * CHUNK)
            for s0 in range(0, rows, split):
                s1 = min(s0 + split, rows)
                nc.gpsimd.dma_start(out=lhs_sb[:, s0:s1], in_=lhst[:, s0:s1])

            with tc.For_i(0, repeats, 1):
                for c in range(nchunk):
                    ps = ppool.tile([CHUNK, n2], mybir.dt.float32)
                    w = lhs_sb[:, c * CHUNK:(c + 1) * CHUNK]
                    for b in range(nbank):
                        nc.tensor.matmul(
                            ps[:, b * BANK:(b + 1) * BANK],
                            w,
                            rhs_sb[:, b * BANK:(b + 1) * BANK],
                            start=True,
                            stop=True,
                        )
                    ot = opool.tile([CHUNK, n2], mybir.dt.float32)
                    if half > 0:
                        nc.vector.tensor_copy(out=ot[:, :half], in_=ps[:, :half])
                    if half < n2:
                        nc.scalar.copy(out=ot[:, half:], in_=ps[:, half:])
                    nc.sync.dma_start(
                        out=out[c * CHUNK:(c + 1) * CHUNK, :], in_=ot[:]
                    )

            dt = opool.tile([1, 4], mybir.dt.float32)
            nc.vector.tensor_copy(out=dt[:], in_=rhs_sb[:1, :4])
            nc.sync.dma_start(out=tout[:, :], in_=dt[:])

    _split_multi_waits(nc)
    return nc


def _prep_inputs(mat_1, mat_2, rows=ROWS, rows_valid=ROWS_VALID, n2=N2):
    """Host-side: shard + transpose + augment, f32 -> bf16 (hi/lo for norms)."""
    mat_1 = np.ascontiguousarray(np.asarray(mat_1, dtype=np.float32))
    mat_2 = np.ascontiguousarray(np.asarray(mat_2, dtype=np.float32))

    sq1 = np.square(mat_1, dtype=np.float32).sum(axis=1, dtype=np.float32)
    sq2 = np.square(mat_2, dtype=np.float32).sum(axis=1, dtype=np.float32)

    def hi_lo(v):
        hi = v.astype(BF16)
        lo = (v - hi.astype(np.float32)).astype(BF16)
        return hi, lo

    hi1, lo1 = hi_lo(sq1)
    hi2, lo2 = hi_lo(sq2)

    rhs = np.zeros((K, n2), dtype=BF16)
    rhs[0:D] = (-2.0 * mat_2.T).astype(BF16)
    rhs[D] = 1
    rhs[D + 1] = 1
    rhs[D + 2] = hi2
    rhs[D + 3] = lo2

    in_maps = []
    for c in range(NCORES):
        sl = slice(c * rows_valid, (c + 1) * rows_valid)
        lt = np.zeros((K, rows), dtype=BF16)
        lt[0:D, :rows_valid] = mat_1[sl].T.astype(BF16)
        lt[D, :rows_valid] = hi1[sl]
        lt[D + 1, :rows_valid] = lo1[sl]
        lt[D + 2] = 1
        lt[D + 3] = 1
        in_maps.append({"lhst": lt, "rhs": rhs})
    return in_maps


def kernel(mat_1, mat_2):
    if "nc" not in _CACHE:
        _CACHE["nc"] = build_nc()
    nc = _CACHE["nc"]
    in_maps = _prep_inputs(mat_1, mat_2)
    res = run_bass_kernel_spmd(nc, in_maps, core_ids=list(range(NCORES)))
    return np.concatenate(
        [res.results[c]["out"][:ROWS_VALID] for c in range(NCORES)], axis=0
    )
